# revision 44
# baseline (speedup 1.0000x reference)
"""Trainium2 Bass kernel for BasicAttention (fp16 pipeline).

reference (per batch b):
    e        = context @ question^T          # [Lc, Lq]
    attn     = softmax(e, axis=-1)
    attn_out = attn @ question               # [Lc, D]
    out      = concat([context, attn_out], -1)  # [Lc, 2D]

Shapes: B=16, Lq=512, Lc=2048, D=1024, fp32 I/O.
Sharding: data-parallel over batch. 8 cores x 2 batches each.

Cost-model floors per core: DMA = 44.0 MB / 360 GB/s = 122.4 us
(C fp16-cast 8.4 MB + Q fp16-cast 2.1 MB + ctx D2D 16.8 MB + ao 16.8 MB);
PE = 319488 cyc @ 2.4 GHz = 133.1 us (MM1+MM2 at 1.0 cyc/row fp16, all
transposes 1.0 cyc/row). fp16 operands (vs fp32's 4.0 / f32r's 1.0
matmul + 1.5 transpose cyc/row and 4-byte loads) pass the 2e-2 gate with
~2x margin: measured 1.03e-2 on HW, dominated by fp16 logit noise
amplified at near-tie softmax rows; bf16 logits fail (8.7e-2), fp8 MM2
fails (4.6e-2), so fp16 is the unique sweet spot. Timeline-sim:
144.2 us vs the 167.8 us f32r baseline.

Per-core pipeline, flat tile sequence t = b*n_ct + i; per iteration:
softmax(t+1), C-load+transpose(t+pre), ctx-copy(t), MM1(t+2),
MM2+store(t). MM1 two tiles ahead / softmax one ahead keeps the serial
exp -> pt-transpose -> MM2 chain off the PE critical path. Per tile:
  - SWDGE cast-load C tile [128c, 1024d] fp32->fp16 (Pool desc-gen,
    2048B bursts, half the DMA bytes of fp32)
  - ctx half of the output: DRAM->DRAM DMA, also on the Pool SWDGE ring:
    its in-order stream paces one ctx copy per iteration right behind
    that iteration's C load (an HWDGE ring would race ahead and flood
    the shared DMA engines, starving the PE of C tiles)
  - PE-transpose C tile -> Ct [d, c] fp16 (fp16 PSUM, DVE+ACT evicts)
  - MM1 (fp16, 1.0 cyc/row): e_psum[128c, 512q] = sum_dj Ct_dj^T . Qt_dj
  - softmax over free dim q: DVE reduce_max(negate) -> ACT exp(bias=-max,
    accum_out=rowsum) -> DVE reciprocal (p unnormalized fp16)
  - PE-transpose p -> pt [q, c] fp16
  - MM2 (fp16): ao_psum[128c, 512d] = sum_qj pt_qj^T . Qnat_qj. The two
    n-half accumulation groups run nh-OUTER (LD_WEIGHTS is free), so
    half 0 evicts (ACT Copy(scale=1/rowsum)) at MM2's midpoint, halving
    ps_ao PSUM-bank hold time; half 1 evicts on DVE -> fp32
  - DMA out attn_out half fp32 (ACT HWDGE ring); in the last K_STAIL
    tiles each half stores separately (ACT + SP rings) right after its
    eviction so the final stores face a shallow DMA-engine queue
Q is SWDGE cast-loaded (fp32 -> fp16) per batch and PE-transposed into
Qt [d, q]; the head interleaves Qh0, C0, C1, Qh1 on the Pool desc-gen
chain (994 ns fixed cost each) so qt/Ct/MM1 chase the landing order,
with identity warm-up transposes (identity itself is a NEFF-embedded
const DMA) covering the p-state ramp. The next batch's Q load and qt
build are split across the previous batch's tail iterations, with the
qt evictions routed entirely to DVE there so the in-flight tiles' exp
is never queued behind them on ACT. The last
tile runs its MM2 nh-outer so eviction+store of half 0 overlap the
half-1 accumulation. SWDGE descriptor ring is enlarged (K_SCRATCH) so
Q (512 descs) + C (128 descs each) desc-gen never stalls on drain.
"""

import sys

if "/opt/trn_rl_repo" not in sys.path:
    sys.path.insert(0, "/opt/trn_rl_repo")

import numpy as np

B = 16
LQ = 512
LC = 2048
D = 1024
N_CORES = 8
B_PER_CORE = B // N_CORES  # 2
NQ = LQ // 128  # 4
ND = D // 128  # 8
NCT = LC // 128  # 16

_CACHE = {}


def _emit(nc, tc, q_ap, c_ap, out_ap, ctx):
    import os

    import concourse.mybir as mybir

    n_b = int(os.environ.get("K_NB", B_PER_CORE))
    n_ct = int(os.environ.get("K_NCT", NCT))

    f32 = mybir.dt.float32
    f16 = mybir.dt.float16
    Exp = mybir.ActivationFunctionType.Exp
    Copy = mybir.ActivationFunctionType.Copy
    X = mybir.AxisListType.X

    def _bufs(name, default):
        return int(os.environ.get(f"K_BUFS_{name}", default))

    const_pool = ctx.enter_context(tc.tile_pool(name="const", bufs=1))
    qnat_pool = ctx.enter_context(tc.tile_pool(name="qnat", bufs=2))
    qt_pool = ctx.enter_context(tc.tile_pool(name="qt", bufs=2))
    cnat_pool = ctx.enter_context(tc.tile_pool(name="cnat", bufs=_bufs("cnat", 6)))
    ct_pool = ctx.enter_context(tc.tile_pool(name="ct", bufs=_bufs("ct", 6)))
    p_pool = ctx.enter_context(tc.tile_pool(name="p", bufs=_bufs("p", 3)))
    pt_pool = ctx.enter_context(tc.tile_pool(name="pt", bufs=_bufs("pt", 3)))
    ao_pool = ctx.enter_context(tc.tile_pool(name="ao", bufs=_bufs("ao", 4)))
    stat_pool = ctx.enter_context(tc.tile_pool(name="stat", bufs=_bufs("stat", 8)))
    ps_tr = ctx.enter_context(
        tc.tile_pool(name="ps_tr", bufs=_bufs("pstr", 4), space="PSUM")
    )
    ps_e = ctx.enter_context(
        tc.tile_pool(name="ps_e", bufs=_bufs("pse", 2), space="PSUM")
    )
    ps_ao = ctx.enter_context(
        tc.tile_pool(name="ps_ao", bufs=_bufs("psao", 2), space="PSUM")
    )

    qnats = {}

    state = {}

    def emit_qnat(b):
        if b in qnats:
            return
        qn = qnat_pool.tile([128, NQ, D], f16, tag="qnat")
        # two half-loads (d-split) so the first Qt transposes can start
        # before the whole Q tensor has landed; SWDGE casts fp32 -> fp16
        qsrc = q_ap[b].rearrange("(a p) d -> p a d", p=128)
        nc.gpsimd.dma_start(qn[:, :, 0 : D // 2], qsrc[:, :, 0 : D // 2])
        nc.gpsimd.dma_start(qn[:, :, D // 2 : D], qsrc[:, :, D // 2 : D])
        qnats[b] = qn

    def q_transposes_part(qnat, qt, djs, evict="alt"):
        # evict: "alt" = DVE/ACT alternating (head, both engines idle);
        # "dve" = all DVE (batch boundary: keeps the ACT stream clear so
        # the in-flight tile's exp isn't delayed behind qt evictions)
        for dj in djs:
            ps = ps_tr.tile([128, 512], f16, tag="ps_tr")
            for qi in range(NQ):
                nc.tensor.transpose(
                    ps[:, qi * 128 : (qi + 1) * 128],
                    qnat[:, qi, dj * 128 : (dj + 1) * 128],
                    ident[:],
                )
            if evict == "dve" or dj % 2 == 0:
                nc.vector.tensor_copy(qt[:, dj, :], ps[:])
            else:
                nc.scalar.copy(qt[:, dj, :], ps[:])
        return qt

    def q_transposes(qnat, evict="alt"):
        qt = qt_pool.tile([128, ND, LQ], f16, tag="qt")
        return q_transposes_part(qnat, qt, range(ND), evict)

    def load_c(b, i):
        """SWDGE cast-DMA C tile i (fp32 -> fp16)."""
        cs = slice(i * 128, (i + 1) * 128)
        cnat = cnat_pool.tile([128, D], f16, tag="cnat")
        state["last_load"] = nc.gpsimd.dma_start(cnat[:], c_ap[b, cs, :])
        return cnat

    def transpose_c(cnat):
        ct = ct_pool.tile([128, D], f16, tag="ct")
        for half in range(2):
            ps = ps_tr.tile([128, 512], f16, tag="ps_tr")
            for k in range(4):
                dj = half * 4 + k
                nc.tensor.transpose(
                    ps[:, k * 128 : (k + 1) * 128],
                    cnat[:, dj * 128 : (dj + 1) * 128],
                    ident[:],
                )
            if half == 0:
                nc.vector.tensor_copy(ct[:, 0:512], ps[:])
            else:
                nc.scalar.copy(ct[:, 512:1024], ps[:])
        return ct

    def load_and_transpose(b, i):
        cnat = load_c(b, i)
        return cnat, transpose_c(cnat)

    def ctx_copy(b, i):
        # context half of the output: DRAM->DRAM. Issued on the Pool SWDGE
        # ring (not an HWDGE ring): SP would race ahead and flood the shared
        # DMA engines with ctx transfers before the Q/C loads; Pool's
        # in-order stream paces one ctx copy per iteration, right behind
        # that iteration's C-tile load
        cs = slice(i * 128, (i + 1) * 128)
        nc.gpsimd.dma_start(out_ap[b, cs, 0:D], c_ap[b, cs, :])

    def mm1(qt, ct):
        e_ps = ps_e.tile([128, 512], f32, tag="e")
        for dj in range(ND):
            nc.tensor.matmul(
                e_ps[:],
                ct[:, dj * 128 : (dj + 1) * 128],
                qt[:, dj, :],
                start=(dj == 0),
                stop=(dj == ND - 1),
            )
        return e_ps

    def softmax(e_ps):
        negmax = stat_pool.tile([128, 1], f32, tag="negmax")
        nc.vector.reduce_max(negmax[:], e_ps[:], axis=X, negate=True)
        p = p_pool.tile([128, LQ], f16, tag="p")
        sumexp = stat_pool.tile([128, 1], f32, tag="sumexp")
        nc.scalar.activation(
            p[:], e_ps[:], Exp, bias=negmax[:], scale=1.0, accum_out=sumexp[:]
        )
        return p, sumexp

    def mm2_and_store(b, qnat, i, p, sumexp):
        cs = slice(i * 128, (i + 1) * 128)
        # near the global tail, shift ACT-side copies/evictions to DVE so the
        # final tile's exp isn't queued behind them in the ACT stream
        tailish = b == n_b - 1 and i >= n_ct - 2
        # reciprocal emitted here (not in softmax) so the DVE stream never
        # stalls on exp completion ahead of the next tile's evictions
        recip = stat_pool.tile([128, 1], f32, tag="recip")
        nc.vector.reciprocal(recip[:], sumexp[:])
        pt_ps = ps_tr.tile([128, 512], f16, tag="ps_tr")
        for qj in range(NQ):
            nc.tensor.transpose(
                pt_ps[:, qj * 128 : (qj + 1) * 128],
                p[:, qj * 128 : (qj + 1) * 128],
                ident[:],
            )
        pt = pt_pool.tile([128, LQ], f16, tag="pt")
        if tailish:
            nc.vector.tensor_copy(pt[:], pt_ps[:])
        else:
            nc.scalar.copy(pt[:], pt_ps[:])

        ao = ao_pool.tile([128, D], f32, tag="ao")
        ao_ps = [
            ps_ao.tile([128, 512], f32, tag="ao", name=f"ao_ps{nh}")
            for nh in range(2)
        ]
        last = b == n_b - 1 and i == n_ct - 1
        if last:
            # drain the kernel tail: nh-outer groups, evicting and storing
            # half 0 while half 1 still accumulates (LD_WEIGHTS is free);
            # the final half goes out in 256-wide pieces so the last
            # DMA's eviction+dge+transfer chain is as short as possible
            for nh in range(2):
                for qj in range(NQ):
                    nc.tensor.matmul(
                        ao_ps[nh][:],
                        pt[:, qj * 128 : (qj + 1) * 128],
                        qnat[:, qj, nh * 512 : (nh + 1) * 512],
                        start=(qj == 0),
                        stop=(qj == NQ - 1),
                    )
                if nh == 0:
                    nc.scalar.activation(
                        ao[:, 0:512], ao_ps[0][:], Copy, scale=recip[:]
                    )
                    nc.scalar.dma_start(out_ap[b, cs, D : D + 512], ao[:, 0:512])
                else:
                    nc.vector.tensor_scalar_mul(
                        ao[:, 512:1024], ao_ps[1][:], recip[:]
                    )
                    nc.sync.dma_start(
                        out_ap[b, cs, D + 512 : 2 * D], ao[:, 512:1024]
                    )
            return
        # qj-outer: both n-half accumulation groups consume the same
        # stationary pt tile back-to-back (one weight load feeds two MMs)
        for qj in range(NQ):
            for nh in range(2):
                nc.tensor.matmul(
                    ao_ps[nh][:],
                    pt[:, qj * 128 : (qj + 1) * 128],
                    qnat[:, qj, nh * 512 : (nh + 1) * 512],
                    start=(qj == 0),
                    stop=(qj == NQ - 1),
                )
        # split the two normalizing evictions across ACT and DVE so neither
        # engine queues two 0.5us copies between consecutive softmax ops
        nc.scalar.activation(
            ao[:, 0:512], ao_ps[0][:], Copy, scale=recip[:]
        )
        nc.vector.tensor_scalar_mul(ao[:, 512:1024], ao_ps[1][:], recip[:])
        nc.scalar.dma_start(out_ap[b, cs, D : 2 * D], ao[:])

    # Software pipeline over the flat tile sequence t = b*n_ct + i.
    # Per iteration t the emission is: softmax(t+1), C-load+transpose(t+pre),
    # ctx copy(t), MM1(t+2), MM2+store(t). Running MM1 two tiles ahead and
    # softmax one tile ahead means ACT's exp(t+1) is complete before the PE
    # reaches pt-transpose(t+1), so the serial exp->pt->MM2 chain never
    # stalls the PE. Each batch's Q DMA + Qt build is emitted during the
    # previous batch's tail so the batch boundary has no bubble.
    pre = int(os.environ.get("K_PRE", 5))
    pre = max(2, min(pre, n_ct))
    T = n_b * n_ct
    # head DMA order Qh0, C0, C1, Qh1, C2, ...: the Pool desc-gen chain
    # (994 ns fixed each) is the head's serial resource, so the first Q half
    # (which gates qt dj0-3 and MM1) goes first, then the first C tiles
    # interleave ahead of the second half
    # identity comes as a NEFF-embedded const via one HWDGE DMA (~1.4us
    # ready) instead of a Pool-engine memset/iota/affine_select chain
    # (~2.6us) that would also delay the head Q/C desc-gens
    ident_dram = nc.inline_tensor(np.eye(128, dtype=np.float16), name="ident16")
    ident = const_pool.tile([128, 128], f16, tag="ident_f16")
    nc.sync.dma_start(ident[:], ident_dram.ap())

    qnat_head = qnat_pool.tile([128, NQ, D], f16, tag="qnat")
    qsrc0 = q_ap[0].rearrange("(a p) d -> p a d", p=128)
    head_n = min(pre, T)
    head_cnats = {}
    nc.gpsimd.dma_start(qnat_head[:, :, 0 : D // 2], qsrc0[:, :, 0 : D // 2])
    for k in range(min(2, head_n)):
        head_cnats[k] = load_c(0, k)
    nc.gpsimd.dma_start(qnat_head[:, :, D // 2 : D], qsrc0[:, :, D // 2 : D])
    for k in range(2, head_n):
        head_cnats[k] = load_c(0, k)
    qnats[0] = qnat_head

    # HAM pre-warm: dummy matmuls on the identity fill the otherwise-idle
    # first-load window so the first real transposes/matmuls run ramped
    n_warm = int(os.environ.get("K_WARM", 4))
    if n_warm:
        warm_ps = ps_tr.tile([128, 512], f16, tag="ps_tr", name="warm_ps")
        for w in range(n_warm):
            for qi in range(NQ):
                nc.tensor.transpose(
                    warm_ps[:, qi * 128 : (qi + 1) * 128], ident[:], ident[:]
                )

    qts = {0: q_transposes(qnat_head)}
    lts = {t: (cn, transpose_c(cn)) for t, cn in head_cnats.items()}

    def qt_of(t):
        return qts[t // n_ct]

    es = {0: mm1(qt_of(0), lts[0][1])}
    if T > 1:
        es[1] = mm1(qt_of(1), lts[1][1])
    sms = {0: softmax(es[0])}

    for t in range(T):
        b, i = divmod(t, n_ct)
        if t + 1 < T:
            sms[t + 1] = softmax(es[t + 1])
        adv = int(os.environ.get("K_CTXADV", 3))
        if t < T - adv:
            ctx_copy(b, i)
            if t == T - adv - 1:
                for tt in range(T - adv, T):
                    ctx_copy(*divmod(tt, n_ct))
        qlead = int(os.environ.get("K_QLEAD", pre))
        if t + qlead < T and (t + qlead) % n_ct == 0:
            emit_qnat((t + qlead) // n_ct)
        if t + pre < T:
            b2, i2 = divmod(t + pre, n_ct)
            if i2 == 0:
                emit_qnat(b2)
            lts[t + pre] = load_and_transpose(b2, i2)
            # qt(b2) must exist by iter b2*n_ct - 2 (MM1 runs 2 ahead);
            # built in two halves to smooth the PE spike at the boundary
            if b2 not in qts and i2 == pre - 3:
                qts[b2] = q_transposes_part(
                    qnats[b2],
                    qt_pool.tile([128, ND, LQ], f16, tag="qt", name="qt_b"),
                    range(ND // 2),
                    evict="dve",
                )
            elif b2 in qts and i2 == pre - 2 and b2 * n_ct > t:
                q_transposes_part(
                    qnats[b2], qts[b2], range(ND // 2, ND), evict="dve"
                )
        if t + 2 < T:
            es[t + 2] = mm1(qt_of(t + 2), lts[t + 2][1])
            del lts[t + 2]
        p, sumexp = sms.pop(t)
        es.pop(t)
        mm2_and_store(b, qnats[b], i, p, sumexp)


def _build():
    if "nc" in _CACHE:
        return _CACHE["nc"]
    from contextlib import ExitStack

    import concourse.bacc as bacc
    import concourse.mybir as mybir
    import concourse.tile as tile

    f32 = mybir.dt.float32
    import os

    nc = bacc.Bacc(
        "TRN2",
        target_bir_lowering=False,
        debug=False,
        # SWDGE descriptor ring: Q cast-loads are 512 descs each and C
        # cast-loads 128 each; the default 1024-desc ring stalls desc-gen
        # behind transfer drain and starves the PE of C tiles
        dynamic_dma_scratch_size=int(os.environ.get("K_SCRATCH", 131072)),
    )
    q = nc.dram_tensor("q", [B_PER_CORE, LQ, D], f32, kind="ExternalInput").ap()
    c = nc.dram_tensor("c", [B_PER_CORE, LC, D], f32, kind="ExternalInput").ap()
    out = nc.dram_tensor(
        "out", [B_PER_CORE, LC, 2 * D], f32, kind="ExternalOutput"
    ).ap()
    with tile.TileContext(nc) as tc:
        with ExitStack() as ctx:
            _emit(nc, tc, q, c, out, ctx)
    nc.compile()
    _CACHE["nc"] = nc
    return nc


def kernel(question, context):
    from concourse import bass_utils

    nc = _build()
    question = np.ascontiguousarray(question, dtype=np.float32)
    context = np.ascontiguousarray(context, dtype=np.float32)
    in_maps = [
        {
            "q": question[i * B_PER_CORE : (i + 1) * B_PER_CORE],
            "c": context[i * B_PER_CORE : (i + 1) * B_PER_CORE],
        }
        for i in range(N_CORES)
    ]
    res = bass_utils.run_bass_kernel_spmd(nc, in_maps, core_ids=list(range(N_CORES)))
    return np.concatenate([res.results[i]["out"] for i in range(N_CORES)], axis=0)


# revision 49
# speedup vs baseline: 1.0013x; 1.0013x over previous
"""Trainium2 Bass kernel for BasicAttention (fp16 pipeline).

reference (per batch b):
    e        = context @ question^T          # [Lc, Lq]
    attn     = softmax(e, axis=-1)
    attn_out = attn @ question               # [Lc, D]
    out      = concat([context, attn_out], -1)  # [Lc, 2D]

Shapes: B=16, Lq=512, Lc=2048, D=1024, fp32 I/O.
Sharding: data-parallel over batch. 8 cores x 2 batches each.

Cost-model floors per core: DMA = 44.0 MB / 360 GB/s = 122.4 us
(C fp16-cast 8.4 MB + Q fp16-cast 2.1 MB + ctx D2D 16.8 MB + ao 16.8 MB);
PE = 319488 cyc @ 2.4 GHz = 133.1 us (MM1+MM2 at 1.0 cyc/row fp16, all
transposes 1.0 cyc/row). fp16 operands (vs fp32's 4.0 / f32r's 1.0
matmul + 1.5 transpose cyc/row and 4-byte loads) pass the 2e-2 gate with
~2x margin: measured 1.03e-2 on HW, dominated by fp16 logit noise
amplified at near-tie softmax rows; bf16 logits fail (8.7e-2), fp8 MM2
fails (4.6e-2), so fp16 is the unique sweet spot. Timeline-sim:
144.0 us vs the 167.8 us f32r baseline.

Per-core pipeline, flat tile sequence t = b*n_ct + i; per iteration:
softmax(t+1), C-load+transpose(t+pre), ctx-copy(t), MM1(t+2),
MM2+store(t). MM1 two tiles ahead / softmax one ahead keeps the serial
exp -> pt-transpose -> MM2 chain off the PE critical path. Per tile:
  - SWDGE cast-load C tile [128c, 1024d] fp32->fp16 (Pool desc-gen,
    2048B bursts, half the DMA bytes of fp32)
  - ctx half of the output: DRAM->DRAM DMA, also on the Pool SWDGE ring:
    its in-order stream paces one ctx copy per iteration right behind
    that iteration's C load (an HWDGE ring would race ahead and flood
    the shared DMA engines, starving the PE of C tiles)
  - PE-transpose C tile -> Ct [d, c] fp16 (fp16 PSUM, DVE+ACT evicts)
  - MM1 (fp16, 1.0 cyc/row): e_psum[128c, 512q] = sum_dj Ct_dj^T . Qt_dj
  - softmax over free dim q: DVE reduce_max(negate) -> ACT exp(bias=-max,
    accum_out=rowsum) -> DVE reciprocal (p unnormalized fp16)
  - PE-transpose p -> pt [q, c] fp16
  - MM2 (fp16): ao_psum[128c, 512d] = sum_qj pt_qj^T . Qnat_qj. The two
    n-half accumulation groups run nh-OUTER (LD_WEIGHTS is free), so
    half 0 evicts (ACT Copy(scale=1/rowsum)) at MM2's midpoint, halving
    ps_ao PSUM-bank hold time; half 1 evicts on DVE -> fp32
  - DMA out attn_out half fp32 (ACT HWDGE ring); in the last K_STAIL
    tiles each half stores separately (ACT + SP rings) right after its
    eviction so the final stores face a shallow DMA-engine queue
Q is SWDGE cast-loaded (fp32 -> fp16) per batch and PE-transposed into
Qt [d, q]; the head interleaves Qh0, C0, C1, Qh1 on the Pool desc-gen
chain (994 ns fixed cost each) so qt/Ct/MM1 chase the landing order,
with identity warm-up transposes (identity itself is a NEFF-embedded
const DMA) covering the p-state ramp. The next batch's Q load and qt
build are split across the previous batch's tail iterations, with the
qt evictions routed entirely to DVE there so the in-flight tiles' exp
is never queued behind them on ACT. The last
tile runs its MM2 nh-outer so eviction+store of half 0 overlap the
half-1 accumulation. SWDGE descriptor ring is enlarged (K_SCRATCH) so
Q (512 descs) + C (128 descs each) desc-gen never stalls on drain.
"""

import sys

if "/opt/trn_rl_repo" not in sys.path:
    sys.path.insert(0, "/opt/trn_rl_repo")

import numpy as np

B = 16
LQ = 512
LC = 2048
D = 1024
N_CORES = 8
B_PER_CORE = B // N_CORES  # 2
NQ = LQ // 128  # 4
ND = D // 128  # 8
NCT = LC // 128  # 16

_CACHE = {}


def _emit(nc, tc, q_ap, c_ap, out_ap, ctx):
    import os

    import concourse.mybir as mybir

    n_b = int(os.environ.get("K_NB", B_PER_CORE))
    n_ct = int(os.environ.get("K_NCT", NCT))

    f32 = mybir.dt.float32
    f16 = mybir.dt.float16
    Exp = mybir.ActivationFunctionType.Exp
    Copy = mybir.ActivationFunctionType.Copy
    X = mybir.AxisListType.X

    def _bufs(name, default):
        return int(os.environ.get(f"K_BUFS_{name}", default))

    const_pool = ctx.enter_context(tc.tile_pool(name="const", bufs=1))
    qnat_pool = ctx.enter_context(tc.tile_pool(name="qnat", bufs=2))
    qt_pool = ctx.enter_context(tc.tile_pool(name="qt", bufs=2))
    cnat_pool = ctx.enter_context(tc.tile_pool(name="cnat", bufs=_bufs("cnat", 6)))
    ct_pool = ctx.enter_context(tc.tile_pool(name="ct", bufs=_bufs("ct", 6)))
    p_pool = ctx.enter_context(tc.tile_pool(name="p", bufs=_bufs("p", 3)))
    pt_pool = ctx.enter_context(tc.tile_pool(name="pt", bufs=_bufs("pt", 3)))
    ao_pool = ctx.enter_context(tc.tile_pool(name="ao", bufs=_bufs("ao", 4)))
    stat_pool = ctx.enter_context(tc.tile_pool(name="stat", bufs=_bufs("stat", 8)))
    ps_tr = ctx.enter_context(
        tc.tile_pool(name="ps_tr", bufs=_bufs("pstr", 4), space="PSUM")
    )
    ps_e = ctx.enter_context(
        tc.tile_pool(name="ps_e", bufs=_bufs("pse", 2), space="PSUM")
    )
    ps_ao = ctx.enter_context(
        tc.tile_pool(name="ps_ao", bufs=_bufs("psao", 2), space="PSUM")
    )

    qnats = {}

    state = {}

    def emit_qnat(b):
        if b in qnats:
            return
        qn = qnat_pool.tile([128, NQ, D], f16, tag="qnat")
        # two half-loads (d-split) so the first Qt transposes can start
        # before the whole Q tensor has landed; SWDGE casts fp32 -> fp16
        qsrc = q_ap[b].rearrange("(a p) d -> p a d", p=128)
        nc.gpsimd.dma_start(qn[:, :, 0 : D // 2], qsrc[:, :, 0 : D // 2])
        nc.gpsimd.dma_start(qn[:, :, D // 2 : D], qsrc[:, :, D // 2 : D])
        qnats[b] = qn

    def q_transposes_part(qnat, qt, djs, evict="alt"):
        # evict: "alt" = DVE/ACT alternating (head, both engines idle);
        # "dve" = all DVE (batch boundary: keeps the ACT stream clear so
        # the in-flight tile's exp isn't delayed behind qt evictions)
        for dj in djs:
            ps = ps_tr.tile([128, 512], f16, tag="ps_tr")
            for qi in range(NQ):
                nc.tensor.transpose(
                    ps[:, qi * 128 : (qi + 1) * 128],
                    qnat[:, qi, dj * 128 : (dj + 1) * 128],
                    ident[:],
                )
            if evict == "dve" or dj % 2 == 0:
                nc.vector.tensor_copy(qt[:, dj, :], ps[:])
            else:
                nc.scalar.copy(qt[:, dj, :], ps[:])
        return qt

    def q_transposes(qnat, evict="alt"):
        qt = qt_pool.tile([128, ND, LQ], f16, tag="qt")
        return q_transposes_part(qnat, qt, range(ND), evict)

    def load_c(b, i):
        """SWDGE cast-DMA C tile i (fp32 -> fp16)."""
        cs = slice(i * 128, (i + 1) * 128)
        cnat = cnat_pool.tile([128, D], f16, tag="cnat")
        state["last_load"] = nc.gpsimd.dma_start(cnat[:], c_ap[b, cs, :])
        return cnat

    def transpose_c(cnat):
        ct = ct_pool.tile([128, D], f16, tag="ct")
        for half in range(2):
            ps = ps_tr.tile([128, 512], f16, tag="ps_tr")
            for k in range(4):
                dj = half * 4 + k
                nc.tensor.transpose(
                    ps[:, k * 128 : (k + 1) * 128],
                    cnat[:, dj * 128 : (dj + 1) * 128],
                    ident[:],
                )
            if half == 0:
                nc.vector.tensor_copy(ct[:, 0:512], ps[:])
            else:
                nc.scalar.copy(ct[:, 512:1024], ps[:])
        return ct

    def load_and_transpose(b, i):
        cnat = load_c(b, i)
        return cnat, transpose_c(cnat)

    def ctx_copy(b, i):
        # context half of the output: DRAM->DRAM. Issued on the Pool SWDGE
        # ring (not an HWDGE ring): SP would race ahead and flood the shared
        # DMA engines with ctx transfers before the Q/C loads; Pool's
        # in-order stream paces one ctx copy per iteration, right behind
        # that iteration's C-tile load
        cs = slice(i * 128, (i + 1) * 128)
        nc.gpsimd.dma_start(out_ap[b, cs, 0:D], c_ap[b, cs, :])

    def mm1(qt, ct):
        e_ps = ps_e.tile([128, 512], f32, tag="e")
        for dj in range(ND):
            nc.tensor.matmul(
                e_ps[:],
                ct[:, dj * 128 : (dj + 1) * 128],
                qt[:, dj, :],
                start=(dj == 0),
                stop=(dj == ND - 1),
            )
        return e_ps

    def softmax(e_ps):
        negmax = stat_pool.tile([128, 1], f32, tag="negmax")
        nc.vector.reduce_max(negmax[:], e_ps[:], axis=X, negate=True)
        p = p_pool.tile([128, LQ], f16, tag="p")
        sumexp = stat_pool.tile([128, 1], f32, tag="sumexp")
        nc.scalar.activation(
            p[:], e_ps[:], Exp, bias=negmax[:], scale=1.0, accum_out=sumexp[:]
        )
        return p, sumexp

    def mm2_and_store(b, qnat, i, p, sumexp):
        cs = slice(i * 128, (i + 1) * 128)
        # near the global tail, shift ACT-side copies/evictions to DVE so the
        # final tile's exp isn't queued behind them in the ACT stream
        tailish = b == n_b - 1 and i >= n_ct - 2
        # reciprocal emitted here (not in softmax) so the DVE stream never
        # stalls on exp completion ahead of the next tile's evictions
        recip = stat_pool.tile([128, 1], f32, tag="recip")
        nc.vector.reciprocal(recip[:], sumexp[:])
        pt_ps = ps_tr.tile([128, 512], f16, tag="ps_tr")
        for qj in range(NQ):
            nc.tensor.transpose(
                pt_ps[:, qj * 128 : (qj + 1) * 128],
                p[:, qj * 128 : (qj + 1) * 128],
                ident[:],
            )
        pt = pt_pool.tile([128, LQ], f16, tag="pt")
        if tailish:
            nc.vector.tensor_copy(pt[:], pt_ps[:])
        else:
            nc.scalar.copy(pt[:], pt_ps[:])

        ao = ao_pool.tile([128, D], f32, tag="ao")
        ao_ps = [
            ps_ao.tile([128, 512], f32, tag="ao", name=f"ao_ps{nh}")
            for nh in range(2)
        ]
        last = b == n_b - 1 and i == n_ct - 1
        if last:
            # drain the kernel tail: nh-outer groups, evicting and storing
            # half 0 while half 1 still accumulates (LD_WEIGHTS is free);
            # the final half goes out in 256-wide pieces so the last
            # DMA's eviction+dge+transfer chain is as short as possible
            for nh in range(2):
                for qj in range(NQ):
                    nc.tensor.matmul(
                        ao_ps[nh][:],
                        pt[:, qj * 128 : (qj + 1) * 128],
                        qnat[:, qj, nh * 512 : (nh + 1) * 512],
                        start=(qj == 0),
                        stop=(qj == NQ - 1),
                    )
                if nh == 0:
                    nc.scalar.activation(
                        ao[:, 0:512], ao_ps[0][:], Copy, scale=recip[:]
                    )
                    nc.scalar.dma_start(out_ap[b, cs, D : D + 512], ao[:, 0:512])
                else:
                    nc.vector.tensor_scalar_mul(
                        ao[:, 512:1024], ao_ps[1][:], recip[:]
                    )
                    nc.sync.dma_start(
                        out_ap[b, cs, D + 512 : 2 * D], ao[:, 512:1024]
                    )
            return
        # qj-outer: both n-half accumulation groups consume the same
        # stationary pt tile back-to-back (one weight load feeds two MMs)
        for qj in range(NQ):
            for nh in range(2):
                nc.tensor.matmul(
                    ao_ps[nh][:],
                    pt[:, qj * 128 : (qj + 1) * 128],
                    qnat[:, qj, nh * 512 : (nh + 1) * 512],
                    start=(qj == 0),
                    stop=(qj == NQ - 1),
                )
        # split the two normalizing evictions across ACT and DVE so neither
        # engine queues two 0.5us copies between consecutive softmax ops
        nc.scalar.activation(
            ao[:, 0:512], ao_ps[0][:], Copy, scale=recip[:]
        )
        nc.vector.tensor_scalar_mul(ao[:, 512:1024], ao_ps[1][:], recip[:])
        nc.scalar.dma_start(out_ap[b, cs, D : 2 * D], ao[:])

    # Software pipeline over the flat tile sequence t = b*n_ct + i.
    # Per iteration t the emission is: softmax(t+1), C-load+transpose(t+pre),
    # ctx copy(t), MM1(t+2), MM2+store(t). Running MM1 two tiles ahead and
    # softmax one tile ahead means ACT's exp(t+1) is complete before the PE
    # reaches pt-transpose(t+1), so the serial exp->pt->MM2 chain never
    # stalls the PE. Each batch's Q DMA + Qt build is emitted during the
    # previous batch's tail so the batch boundary has no bubble.
    pre = int(os.environ.get("K_PRE", 5))
    pre = max(2, min(pre, n_ct))
    T = n_b * n_ct
    # head DMA order Qh0, C0, C1, Qh1, C2, ...: the Pool desc-gen chain
    # (994 ns fixed each) is the head's serial resource, so the first Q half
    # (which gates qt dj0-3 and MM1) goes first, then the first C tiles
    # interleave ahead of the second half
    # identity comes as a NEFF-embedded const via one HWDGE DMA (~1.4us
    # ready) instead of a Pool-engine memset/iota/affine_select chain
    # (~2.6us) that would also delay the head Q/C desc-gens
    ident_dram = nc.inline_tensor(np.eye(128, dtype=np.float16), name="ident16")
    ident = const_pool.tile([128, 128], f16, tag="ident_f16")
    nc.sync.dma_start(ident[:], ident_dram.ap())

    qnat_head = qnat_pool.tile([128, NQ, D], f16, tag="qnat")
    qsrc0 = q_ap[0].rearrange("(a p) d -> p a d", p=128)
    head_n = min(pre, T)
    head_cnats = {}
    nc.gpsimd.dma_start(qnat_head[:, :, 0 : D // 2], qsrc0[:, :, 0 : D // 2])
    for k in range(min(2, head_n)):
        head_cnats[k] = load_c(0, k)
    nc.gpsimd.dma_start(qnat_head[:, :, D // 2 : D], qsrc0[:, :, D // 2 : D])
    for k in range(2, head_n):
        head_cnats[k] = load_c(0, k)
    qnats[0] = qnat_head

    # HAM pre-warm: dummy matmuls on the identity fill the otherwise-idle
    # first-load window so the first real transposes/matmuls run ramped
    n_warm = int(os.environ.get("K_WARM", 4))
    if n_warm:
        warm_ps = ps_tr.tile([128, 512], f16, tag="ps_tr", name="warm_ps")
        for w in range(n_warm):
            for qi in range(NQ):
                nc.tensor.transpose(
                    warm_ps[:, qi * 128 : (qi + 1) * 128], ident[:], ident[:]
                )

    qts = {0: q_transposes(qnat_head)}
    lts = {t: (cn, transpose_c(cn)) for t, cn in head_cnats.items()}

    def qt_of(t):
        return qts[t // n_ct]

    es = {0: mm1(qt_of(0), lts[0][1])}
    if T > 1:
        es[1] = mm1(qt_of(1), lts[1][1])
    sms = {0: softmax(es[0])}

    for t in range(T):
        b, i = divmod(t, n_ct)
        if t + 1 < T:
            sms[t + 1] = softmax(es[t + 1])
        adv = int(os.environ.get("K_CTXADV", 3))
        if t < T - adv:
            ctx_copy(b, i)
            if t == T - adv - 1:
                for tt in range(T - adv, T):
                    ctx_copy(*divmod(tt, n_ct))
        qlead = int(os.environ.get("K_QLEAD", pre))
        if t + qlead < T and (t + qlead) % n_ct == 0:
            emit_qnat((t + qlead) // n_ct)
        if t + pre < T:
            b2, i2 = divmod(t + pre, n_ct)
            if i2 == 0:
                emit_qnat(b2)
            lts[t + pre] = load_and_transpose(b2, i2)
            # qt(b2) must exist by iter b2*n_ct - 2 (MM1 runs 2 ahead);
            # built in two halves to smooth the PE spike at the boundary
            if b2 not in qts and i2 == pre - 3:
                qts[b2] = q_transposes_part(
                    qnats[b2],
                    qt_pool.tile([128, ND, LQ], f16, tag="qt", name="qt_b"),
                    range(ND // 2),
                    evict="dve",
                )
            elif b2 in qts and i2 == pre - 2 and b2 * n_ct > t:
                q_transposes_part(
                    qnats[b2], qts[b2], range(ND // 2, ND), evict="dve"
                )
        if t + 2 < T:
            es[t + 2] = mm1(qt_of(t + 2), lts[t + 2][1])
            del lts[t + 2]
        p, sumexp = sms.pop(t)
        es.pop(t)
        mm2_and_store(b, qnats[b], i, p, sumexp)


def _build():
    if "nc" in _CACHE:
        return _CACHE["nc"]
    from contextlib import ExitStack

    import concourse.bacc as bacc
    import concourse.mybir as mybir
    import concourse.tile as tile

    f32 = mybir.dt.float32
    import os

    nc = bacc.Bacc(
        "TRN2",
        target_bir_lowering=False,
        debug=False,
        # SWDGE descriptor ring: Q cast-loads are 512 descs each and C
        # cast-loads 128 each; the default 1024-desc ring stalls desc-gen
        # behind transfer drain and starves the PE of C tiles
        dynamic_dma_scratch_size=int(os.environ.get("K_SCRATCH", 131072)),
    )
    q = nc.dram_tensor("q", [B_PER_CORE, LQ, D], f32, kind="ExternalInput").ap()
    c = nc.dram_tensor("c", [B_PER_CORE, LC, D], f32, kind="ExternalInput").ap()
    out = nc.dram_tensor(
        "out", [B_PER_CORE, LC, 2 * D], f32, kind="ExternalOutput"
    ).ap()
    with tile.TileContext(nc) as tc:
        with ExitStack() as ctx:
            _emit(nc, tc, q, c, out, ctx)
    nc.compile()
    _CACHE["nc"] = nc
    return nc


def kernel(question, context):
    from concourse import bass_utils

    nc = _build()
    question = np.ascontiguousarray(question, dtype=np.float32)
    context = np.ascontiguousarray(context, dtype=np.float32)
    in_maps = [
        {
            "q": question[i * B_PER_CORE : (i + 1) * B_PER_CORE],
            "c": context[i * B_PER_CORE : (i + 1) * B_PER_CORE],
        }
        for i in range(N_CORES)
    ]
    res = bass_utils.run_bass_kernel_spmd(nc, in_maps, core_ids=list(range(N_CORES)))
    return np.concatenate([res.results[i]["out"] for i in range(N_CORES)], axis=0)
